# revision 1
# baseline (speedup 1.0000x reference)
"""Trainium2 Bass kernel for nn_DendSeqNetSVHN3 (dendritic LIF sequence net).

Strategy: data-parallel over batch (B=256 -> 32 per NeuronCore x 8 cores).
Per core:
  - inj[t] = einsum(x_t, W_h) + b_h computed on the PE in fp16 with a 3-term
    hi/lo split (x_hi*W_hi + x_lo*W_hi + x_hi*W_lo) for fp32-grade accuracy;
    time is batched into the matmul free dim (chunks of 8 steps).
  - The LIF membrane scan runs on the vector engine with fused
    scalar_tensor_tensor ops on state u = 10*vh_dec in layout
    [128 partitions, 15 j-tiles x 32 batch]; spikes become an fp16 mask.
  - The output stage (summed-spike -> 4 leaky-integrator branches -> sum)
    collapses to one matmul per (chunk, j-tile) against replicated W_o plus
    two linear IIR filters over time, done as tensor_tensor_scan at the end.
  - The response to the constant bias input is added on the host (linearity).
"""
import numpy as np
from contextlib import ExitStack

import concourse.bass as bass
import concourse.mybir as mybir
import concourse.tile as tile
from concourse import bacc
from concourse.bass_utils import run_bass_kernel_spmd

F32 = mybir.dt.float32
F16 = mybir.dt.float16

T, B, NCORES = 100, 256, 8
C, D, H, IN = 3, 3, 200, 1024
NOUT = 10
DHP = 640        # d*h (=600) padded per c
NJ = 15          # (C*DHP)/128 state tiles
NM = 5           # DHP/128 m-tiles per c
NK = 8           # IN/128 k-tiles
BL = B // NCORES # 32 batch per core
NTERMS = 3
TERMS3 = [(0, 0), (1, 0), (0, 1)]   # (x part, w part): hi*Whi + lo*Whi + hi*Wlo
CH = 8           # timesteps per matmul chunk


def _build(T=T, CH=CH, nterms=NTERMS):
    terms = TERMS3[:nterms]
    NX = max(t[0] for t in terms) + 1
    NW = max(t[1] for t in terms) + 1
    NT = T * BL
    chunks = []
    t0 = 0
    while t0 < T:
        tcn = min(CH, T - t0)
        chunks.append((t0, tcn))
        t0 += tcn

    nc = bacc.Bacc("TRN2", target_bir_lowering=False, debug=False)
    xt_d = nc.dram_tensor("xt", [NX, C, IN, NT], F16, kind="ExternalInput").ap()
    wt_d = nc.dram_tensor("wt", [C, NW, IN, DHP], F16, kind="ExternalInput").ap()
    bh_d = nc.dram_tensor("bh", [128, NJ], F32, kind="ExternalInput").ap()
    wmm_d = nc.dram_tensor("wmm", [128, NJ, NOUT], F16, kind="ExternalInput").ap()
    vout_d = nc.dram_tensor("vout", [NOUT, NT], F32, kind="ExternalOutput").ap()

    with tile.TileContext(nc) as tc:
        with ExitStack() as ctx:
            const_p = ctx.enter_context(tc.tile_pool(name="const", bufs=1))
            state_p = ctx.enter_context(tc.tile_pool(name="state", bufs=1))
            xc_p = ctx.enter_context(tc.tile_pool(name="xc", bufs=2))
            injc_p = ctx.enter_context(tc.tile_pool(name="injc", bufs=2))
            maskc_p = ctx.enter_context(tc.tile_pool(name="maskc", bufs=2))
            wtmp_p = ctx.enter_context(tc.tile_pool(name="wtmp", bufs=2))
            psA_p = ctx.enter_context(tc.tile_pool(name="psA", bufs=4, space="PSUM"))
            psP_p = ctx.enter_context(tc.tile_pool(name="psP", bufs=2, space="PSUM"))

            w_sb = const_p.tile([128, C, NW, NK, NM, 128], F16)
            for c in range(C):
                for wi in range(NW):
                    nc.sync.dma_start(
                        w_sb[:, c, wi],
                        wt_d[c, wi].rearrange("(k p) (m q) -> p k m q", p=128, q=128),
                    )
            bh_sb = const_p.tile([128, NJ], F32)
            nc.sync.dma_start(bh_sb[:], bh_d[:])
            wmm_sb = const_p.tile([128, NJ, NOUT], F16)
            nc.sync.dma_start(wmm_sb[:], wmm_d[:])
            dec8_sb = const_p.tile([NOUT, T], F32)
            nc.vector.memset(dec8_sb[:], 0.8)
            dec9_sb = const_p.tile([NOUT, T], F32)
            nc.vector.memset(dec9_sb[:], 0.9)

            u_sb = state_p.tile([128, NJ, BL], F32)
            ih_sb = state_p.tile([128, NJ, BL], F32)
            Pall = state_p.tile([NOUT, NT], F32)
            abuf = state_p.tile([NOUT, NT + BL], F32)
            vout_sb = state_p.tile([NOUT, NT], F32)
            nc.vector.memset(u_sb[:], 0.0)
            nc.vector.memset(ih_sb[:], 0.0)
            nc.vector.memset(abuf[:, 0:BL], 0.0)

            for (t0, tcn) in chunks:
                CW = tcn * BL
                injt = injc_p.tile([128, NJ, CH * BL], F32, tag="injc")
                maskt = maskc_p.tile([128, CH, NJ, BL], F16, tag="maskc")
                for c in range(C):
                    xtile = xc_p.tile([128, NX, NK, CH * BL], F16, tag="xc")
                    for xi in range(NX):
                        nc.sync.dma_start(
                            xtile[:, xi, :, 0:CW],
                            xt_d[xi, c].rearrange("(k p) n -> p k n", p=128)[
                                :, :, t0 * BL : t0 * BL + CW
                            ],
                        )
                    for m in range(NM):
                        ps = psA_p.tile([128, CH * BL], F32, tag="psA")
                        nmm = len(terms) * NK
                        i_mm = 0
                        for (xi, wi) in terms:
                            for k in range(NK):
                                nc.tensor.matmul(
                                    ps[:, 0:CW],
                                    w_sb[:, c, wi, k, m, :],
                                    xtile[:, xi, k, 0:CW],
                                    start=(i_mm == 0),
                                    stop=(i_mm == nmm - 1),
                                )
                                i_mm += 1
                        j = c * NM + m
                        nc.scalar.activation(
                            injt[:, j, 0:CW],
                            ps[:, 0:CW],
                            mybir.ActivationFunctionType.Identity,
                            bias=bh_sb[:, j : j + 1],
                        )
                for tt in range(tcn):
                    inj_sl = injt[:, :, tt * BL : (tt + 1) * BL]
                    nc.vector.scalar_tensor_tensor(
                        ih_sb[:], ih_sb[:], 0.8, inj_sl,
                        mybir.AluOpType.mult, mybir.AluOpType.add,
                    )
                    nc.vector.scalar_tensor_tensor(
                        maskt[:, tt], u_sb[:], 10.0, u_sb[:],
                        mybir.AluOpType.is_gt, mybir.AluOpType.bypass,
                    )
                    w_t = wtmp_p.tile([128, NJ, BL], F32, tag="wtmp")
                    nc.vector.scalar_tensor_tensor(
                        w_t[:], u_sb[:], 10.0, u_sb[:],
                        mybir.AluOpType.is_le, mybir.AluOpType.mult,
                    )
                    nc.vector.scalar_tensor_tensor(
                        u_sb[:], w_t[:], 0.9, ih_sb[:],
                        mybir.AluOpType.mult, mybir.AluOpType.add,
                    )
                psP = psP_p.tile([NOUT, CH * BL], F32, tag="psP")
                for j in range(NJ):
                    nc.tensor.matmul(
                        psP[:, 0:CW],
                        wmm_sb[:, j, :],
                        maskt[:, 0:tcn, j, :],
                        start=(j == 0),
                        stop=(j == NJ - 1),
                    )
                nc.scalar.copy(Pall[:, t0 * BL : t0 * BL + CW], psP[:, 0:CW])

            Pall_bt = Pall.rearrange("n (t b) -> n b t", b=BL)
            aw_bt = abuf[:, BL : BL + NT].rearrange("n (t b) -> n b t", b=BL)
            ar_bt = abuf[:, 0:NT].rearrange("n (t b) -> n b t", b=BL)
            vout_bt = vout_sb.rearrange("n (t b) -> n b t", b=BL)
            for b in range(BL):
                nc.vector.tensor_tensor_scan(
                    aw_bt[:, b], dec8_sb[:], Pall_bt[:, b], 0.0,
                    mybir.AluOpType.mult, mybir.AluOpType.add,
                )
            for b in range(BL):
                nc.vector.tensor_tensor_scan(
                    vout_bt[:, b], dec9_sb[:], ar_bt[:, b], 0.0,
                    mybir.AluOpType.mult, mybir.AluOpType.add,
                )
            nc.sync.dma_start(vout_d[:], vout_sb[:])
    nc.compile()
    return nc


def _prep_weights(W_h, b_h, W_o, b_o, nterms=NTERMS):
    NW = 2 if nterms >= 3 else 1
    W_hi = W_h.astype(np.float16)
    W_lo = (W_h.astype(np.float32) - W_hi.astype(np.float32)).astype(np.float16)
    wt = np.zeros((C, NW, IN, DHP), np.float16)
    for wi, W in enumerate([W_hi, W_lo][:NW]):
        wt[:, wi, :, : D * H] = W.reshape(C, D * H, IN).transpose(0, 2, 1)
    bh = np.zeros((128, NJ), np.float32)
    wmm = np.zeros((128, NJ, NOUT), np.float16)
    bhf = b_h.reshape(C, D * H)
    O = W_o.shape[0]
    K = H // O
    for j in range(NJ):
        c, m = divmod(j, NM)
        for p in range(128):
            dh = m * 128 + p
            if dh < D * H:
                bh[p, j] = bhf[c, dh]
                h = dh % H
                o, k = divmod(h, K)
                wmm[p, j, :] = (0.1 * W_o[o, :, k]).astype(np.float16)
    K_n = (0.1 * b_o.sum(axis=0)).astype(np.float32)
    return wt, bh, wmm, K_n


def _host_A(K_n, T=T):
    aio = np.zeros(NOUT, np.float32)
    avo = np.zeros(NOUT, np.float32)
    A = np.zeros((T, NOUT), np.float32)
    for t in range(T):
        avo = (np.float32(0.9) * avo + aio).astype(np.float32)
        A[t] = avo
        aio = (np.float32(0.8) * aio + K_n).astype(np.float32)
    return A


def _prep_x_core(x_core, nterms=NTERMS):
    Tl = x_core.shape[0]
    NX = 2 if nterms >= 2 else 1
    xf = np.ascontiguousarray(x_core.reshape(Tl, BL, C, IN))
    x_hi = xf.astype(np.float16)
    parts = [x_hi]
    if NX == 2:
        x_lo = (xf - x_hi.astype(np.float32)).astype(np.float16)
        parts.append(x_lo)
    xt = np.empty((NX, C, IN, Tl * BL), np.float16)
    for xi, xp in enumerate(parts):
        xt[xi] = xp.transpose(2, 3, 0, 1).reshape(C, IN, Tl * BL)
    return xt


_CACHED_NC = None


def run_on_device(x, W_h, b_h, W_o, b_o, trace=False):
    global _CACHED_NC
    x = np.asarray(x, np.float32)
    W_h = np.asarray(W_h, np.float32)
    b_h = np.asarray(b_h, np.float32)
    W_o = np.asarray(W_o, np.float32)
    b_o = np.asarray(b_o, np.float32)
    wt, bh, wmm, K_n = _prep_weights(W_h, b_h, W_o, b_o)
    A = _host_A(K_n)
    in_maps = []
    for core in range(NCORES):
        xt = _prep_x_core(x[:, core * BL : (core + 1) * BL])
        in_maps.append({"xt": xt, "wt": wt, "bh": bh, "wmm": wmm})
    if _CACHED_NC is None:
        _CACHED_NC = _build()
    res = run_bass_kernel_spmd(
        _CACHED_NC, in_maps, core_ids=list(range(NCORES)), trace=trace
    )
    out = np.empty((T, B, NOUT), np.float32)
    for core in range(NCORES):
        v = res.results[core]["vout"]
        out[:, core * BL : (core + 1) * BL, :] = (
            v.reshape(NOUT, T, BL).transpose(1, 2, 0)
        )
    out += A[:, None, :]
    return out, res.exec_time_ns


def kernel(x, W_h, b_h, W_o, b_o):
    out, _ = run_on_device(x, W_h, b_h, W_o, b_o, trace=False)
    return out


# revision 3
# speedup vs baseline: 157.6901x; 157.6901x over previous
"""Trainium2 Bass kernel for nn_DendSeqNetSVHN3 (dendritic LIF sequence net).

Strategy: data-parallel over batch (B=256 -> 32 per NeuronCore x 8 cores).
Per core:
  - inj[t] = einsum(x_t, W_h) + b_h computed on the PE in fp16 with a 3-term
    hi/lo split (x_hi*W_hi + x_lo*W_hi + x_hi*W_lo) for fp32-grade accuracy;
    time is batched into the matmul free dim (chunks of 8 steps).
  - The LIF membrane scan runs on the vector engine with fused
    scalar_tensor_tensor ops on state u = 10*vh_dec in layout
    [128 partitions, 15 j-tiles x 32 batch]; spikes become an fp16 mask.
  - The output stage (summed-spike -> 4 leaky-integrator branches -> sum)
    collapses to one matmul per (chunk, j-tile) against replicated W_o plus
    two linear IIR filters over time, done as tensor_tensor_scan at the end.
  - The response to the constant bias input is added on the host (linearity).
"""
import numpy as np
from contextlib import ExitStack

import concourse.bass as bass
import concourse.mybir as mybir
import concourse.tile as tile
from concourse import bacc
from concourse.bass_utils import run_bass_kernel_spmd

F32 = mybir.dt.float32
F16 = mybir.dt.float16

T, B, NCORES = 100, 256, 8
C, D, H, IN = 3, 3, 200, 1024
NOUT = 10
DHP = 640        # d*h (=600) padded per c
NJ = 15          # (C*DHP)/128 state tiles
NM = 5           # DHP/128 m-tiles per c
NK = 8           # IN/128 k-tiles
BL = B // NCORES # 32 batch per core
NTERMS = 3
TERMS3 = [(0, 0), (1, 0), (0, 1)]   # (x part, w part): hi*Whi + lo*Whi + hi*Wlo
CH = 16          # timesteps per matmul chunk


def _build(T=T, CH=CH, nterms=NTERMS):
    terms = TERMS3[:nterms]
    NX = max(t[0] for t in terms) + 1
    NW = max(t[1] for t in terms) + 1
    NT = T * BL
    chunks = []
    t0 = 0
    while t0 < T:
        tcn = min(CH, T - t0)
        chunks.append((t0, tcn))
        t0 += tcn

    nc = bacc.Bacc("TRN2", target_bir_lowering=False, debug=False)
    xt_d = nc.dram_tensor("xt", [NX, C, IN, NT], F16, kind="ExternalInput").ap()
    wt_d = nc.dram_tensor("wt", [C, NW, IN, DHP], F16, kind="ExternalInput").ap()
    bh_d = nc.dram_tensor("bh", [128, NJ], F32, kind="ExternalInput").ap()
    wmm_d = nc.dram_tensor("wmm", [128, NJ, NOUT], F16, kind="ExternalInput").ap()
    vout_d = nc.dram_tensor("vout", [NOUT, NT], F32, kind="ExternalOutput").ap()

    with tile.TileContext(nc) as tc:
        with ExitStack() as ctx:
            const_p = ctx.enter_context(tc.tile_pool(name="const", bufs=1))
            state_p = ctx.enter_context(tc.tile_pool(name="state", bufs=1))
            xc_p = ctx.enter_context(tc.tile_pool(name="xc", bufs=2))
            injc_p = ctx.enter_context(tc.tile_pool(name="injc", bufs=2))
            maskc_p = ctx.enter_context(tc.tile_pool(name="maskc", bufs=1))
            wtmp_p = ctx.enter_context(tc.tile_pool(name="wtmp", bufs=1))
            psA_p = ctx.enter_context(tc.tile_pool(name="psA", bufs=4, space="PSUM"))
            psP_p = ctx.enter_context(tc.tile_pool(name="psP", bufs=2, space="PSUM"))

            w_sb = const_p.tile([128, C, NW, NK, NM, 128], F16)
            for c in range(C):
                for wi in range(NW):
                    nc.sync.dma_start(
                        w_sb[:, c, wi],
                        wt_d[c, wi].rearrange("(k p) (m q) -> p k m q", p=128, q=128),
                    )
            bh_sb = const_p.tile([128, NJ], F32)
            nc.sync.dma_start(bh_sb[:], bh_d[:])
            wmm_sb = const_p.tile([128, NJ, NOUT], F16)
            nc.sync.dma_start(wmm_sb[:], wmm_d[:])
            dec8_sb = const_p.tile([NOUT, T], F32)
            nc.vector.memset(dec8_sb[:], 0.8)
            dec9_sb = const_p.tile([NOUT, T], F32)
            nc.vector.memset(dec9_sb[:], 0.9)

            u_sb = state_p.tile([128, NJ, BL], F32)
            ih_sb = state_p.tile([128, NJ, BL], F32)
            Pall = state_p.tile([NOUT, NT], F32)
            nc.vector.memset(u_sb[:], 0.0)
            nc.vector.memset(ih_sb[:], 0.0)

            for (t0, tcn) in chunks:
                CW = tcn * BL
                injt = injc_p.tile([128, NJ, CH * BL], F32, tag="injc")
                maskt = maskc_p.tile([128, CH, NJ, BL], F16, tag="maskc")
                for c in range(C):
                    xtile = xc_p.tile([128, NX, NK, CH * BL], F16, tag="xc")
                    for xi in range(NX):
                        nc.sync.dma_start(
                            xtile[:, xi, :, 0:CW],
                            xt_d[xi, c].rearrange("(k p) n -> p k n", p=128)[
                                :, :, t0 * BL : t0 * BL + CW
                            ],
                        )
                    for m in range(NM):
                        ps = psA_p.tile([128, CH * BL], F32, tag="psA")
                        nmm = len(terms) * NK
                        i_mm = 0
                        for (xi, wi) in terms:
                            for k in range(NK):
                                nc.tensor.matmul(
                                    ps[:, 0:CW],
                                    w_sb[:, c, wi, k, m, :],
                                    xtile[:, xi, k, 0:CW],
                                    start=(i_mm == 0),
                                    stop=(i_mm == nmm - 1),
                                )
                                i_mm += 1
                        j = c * NM + m
                        nc.scalar.activation(
                            injt[:, j, 0:CW],
                            ps[:, 0:CW],
                            mybir.ActivationFunctionType.Identity,
                            bias=bh_sb[:, j : j + 1],
                        )
                for tt in range(tcn):
                    inj_sl = injt[:, :, tt * BL : (tt + 1) * BL]
                    nc.vector.scalar_tensor_tensor(
                        ih_sb[:], ih_sb[:], 0.8, inj_sl,
                        mybir.AluOpType.mult, mybir.AluOpType.add,
                    )
                    nc.vector.scalar_tensor_tensor(
                        maskt[:, tt], u_sb[:], 10.0, u_sb[:],
                        mybir.AluOpType.is_gt, mybir.AluOpType.bypass,
                    )
                    w_t = wtmp_p.tile([128, NJ, BL], F32, tag="wtmp")
                    nc.vector.scalar_tensor_tensor(
                        w_t[:], u_sb[:], 10.0, u_sb[:],
                        mybir.AluOpType.is_le, mybir.AluOpType.mult,
                    )
                    nc.vector.scalar_tensor_tensor(
                        u_sb[:], w_t[:], 0.9, ih_sb[:],
                        mybir.AluOpType.mult, mybir.AluOpType.add,
                    )
                psP = psP_p.tile([NOUT, CH * BL], F32, tag="psP")
                for j in range(NJ):
                    nc.tensor.matmul(
                        psP[:, 0:CW],
                        wmm_sb[:, j, :],
                        maskt[:, 0:tcn, j, :],
                        start=(j == 0),
                        stop=(j == NJ - 1),
                    )
                nc.scalar.copy(Pall[:, t0 * BL : t0 * BL + CW], psP[:, 0:CW])

            abuf = xc_p.tile([NOUT, NT + BL], F32, tag="xc")
            vout_sb = xc_p.tile([NOUT, NT], F32, tag="xc")
            nc.vector.memset(abuf[:, 0:BL], 0.0)
            Pall_bt = Pall.rearrange("n (t b) -> n b t", b=BL)
            aw_bt = abuf[:, BL : BL + NT].rearrange("n (t b) -> n b t", b=BL)
            ar_bt = abuf[:, 0:NT].rearrange("n (t b) -> n b t", b=BL)
            vout_bt = vout_sb.rearrange("n (t b) -> n b t", b=BL)
            for b in range(BL):
                nc.vector.tensor_tensor_scan(
                    aw_bt[:, b], dec8_sb[:], Pall_bt[:, b], 0.0,
                    mybir.AluOpType.mult, mybir.AluOpType.add,
                )
            for b in range(BL):
                nc.vector.tensor_tensor_scan(
                    vout_bt[:, b], dec9_sb[:], ar_bt[:, b], 0.0,
                    mybir.AluOpType.mult, mybir.AluOpType.add,
                )
            nc.sync.dma_start(vout_d[:], vout_sb[:])
    nc.compile()
    return nc


def _prep_weights(W_h, b_h, W_o, b_o, nterms=NTERMS):
    NW = 2 if nterms >= 3 else 1
    W_hi = W_h.astype(np.float16)
    W_lo = (W_h.astype(np.float32) - W_hi.astype(np.float32)).astype(np.float16)
    wt = np.zeros((C, NW, IN, DHP), np.float16)
    for wi, W in enumerate([W_hi, W_lo][:NW]):
        wt[:, wi, :, : D * H] = W.reshape(C, D * H, IN).transpose(0, 2, 1)
    O = W_o.shape[0]
    K = H // O
    # per-c padded cdh' layout: [c, m*128+p] with dh = m*128+p < 600 valid
    bh_p = np.zeros((C, DHP), np.float32)
    bh_p[:, : D * H] = b_h.reshape(C, D * H)
    bh = bh_p.reshape(C * NM, 128).T.copy()  # [128, NJ]
    h_of_dh = np.arange(D * H) % H
    wz = (0.1 * W_o.transpose(0, 2, 1).reshape(H, NOUT)[h_of_dh]).astype(np.float16)
    wmm_p = np.zeros((C, DHP, NOUT), np.float16)
    wmm_p[:, : D * H] = wz[None]
    wmm = np.ascontiguousarray(
        wmm_p.reshape(C * NM, 128, NOUT).transpose(1, 0, 2)
    )  # [128, NJ, NOUT]
    K_n = (0.1 * b_o.sum(axis=0)).astype(np.float32)
    return wt, bh, wmm, K_n


def _host_A(K_n, T=T):
    aio = np.zeros(NOUT, np.float32)
    avo = np.zeros(NOUT, np.float32)
    A = np.zeros((T, NOUT), np.float32)
    for t in range(T):
        avo = (np.float32(0.9) * avo + aio).astype(np.float32)
        A[t] = avo
        aio = (np.float32(0.8) * aio + K_n).astype(np.float32)
    return A


def _prep_x_core(x_core, nterms=NTERMS):
    Tl = x_core.shape[0]
    NX = 2 if nterms >= 2 else 1
    xf = np.ascontiguousarray(x_core.reshape(Tl, BL, C, IN))
    x_hi = xf.astype(np.float16)
    parts = [x_hi]
    if NX == 2:
        x_lo = (xf - x_hi.astype(np.float32)).astype(np.float16)
        parts.append(x_lo)
    xt = np.empty((NX, C, IN, Tl * BL), np.float16)
    for xi, xp in enumerate(parts):
        xt[xi] = xp.transpose(2, 3, 0, 1).reshape(C, IN, Tl * BL)
    return xt


_CACHED_NC = None


def run_on_device(x, W_h, b_h, W_o, b_o, trace=False):
    global _CACHED_NC
    x = np.asarray(x, np.float32)
    W_h = np.asarray(W_h, np.float32)
    b_h = np.asarray(b_h, np.float32)
    W_o = np.asarray(W_o, np.float32)
    b_o = np.asarray(b_o, np.float32)
    wt, bh, wmm, K_n = _prep_weights(W_h, b_h, W_o, b_o)
    A = _host_A(K_n)
    in_maps = []
    for core in range(NCORES):
        xt = _prep_x_core(x[:, core * BL : (core + 1) * BL])
        in_maps.append({"xt": xt, "wt": wt, "bh": bh, "wmm": wmm})
    if _CACHED_NC is None:
        _CACHED_NC = _build()
    res = run_bass_kernel_spmd(
        _CACHED_NC, in_maps, core_ids=list(range(NCORES)), trace=trace
    )
    out = np.empty((T, B, NOUT), np.float32)
    for core in range(NCORES):
        v = res.results[core]["vout"]
        out[:, core * BL : (core + 1) * BL, :] = (
            v.reshape(NOUT, T, BL).transpose(1, 2, 0)
        )
    out += A[:, None, :]
    return out, res.exec_time_ns


def kernel(x, W_h, b_h, W_o, b_o):
    out, _ = run_on_device(x, W_h, b_h, W_o, b_o, trace=False)
    return out


# revision 7
# speedup vs baseline: 158.2032x; 1.0033x over previous
"""Trainium2 Bass kernel for nn_DendSeqNetSVHN3 (dendritic LIF sequence net).

Strategy: data-parallel over batch (B=256 -> 32 per NeuronCore x 8 cores).
Per core:
  - inj[t] = einsum(x_t, W_h) + b_h computed on the PE in fp16 with a 3-term
    hi/lo split (x_hi*W_hi + x_lo*W_hi + x_hi*W_lo) for fp32-grade accuracy;
    time is batched into the matmul free dim (chunks of 8 steps).
  - The LIF membrane scan runs on the vector engine with fused
    scalar_tensor_tensor ops on state u = 10*vh_dec in layout
    [128 partitions, 15 j-tiles x 32 batch]; spikes become an fp16 mask.
  - The output stage (summed-spike -> 4 leaky-integrator branches -> sum)
    collapses to one matmul per (chunk, j-tile) against replicated W_o plus
    two linear IIR filters over time, done as tensor_tensor_scan at the end.
  - The response to the constant bias input is added on the host (linearity).
"""
import numpy as np
from contextlib import ExitStack

import concourse.bass as bass
import concourse.mybir as mybir
import concourse.tile as tile
from concourse import bacc
from concourse.bass_utils import run_bass_kernel_spmd

F32 = mybir.dt.float32
F16 = mybir.dt.float16

T, B, NCORES = 100, 256, 8
C, D, H, IN = 3, 3, 200, 1024
NOUT = 10
DHP = 640        # d*h (=600) padded per c
NJ = 15          # (C*DHP)/128 state tiles
NM = 5           # DHP/128 m-tiles per c
NK = 8           # IN/128 k-tiles
BL = B // NCORES # 32 batch per core
NTERMS = 3
TERMS3 = [(0, 0), (1, 0), (0, 1)]   # (x part, w part): hi*Whi + lo*Whi + hi*Wlo
CH = 16          # timesteps per matmul chunk


def _build(T=T, CH=CH, nterms=NTERMS):
    terms = TERMS3[:nterms]
    NX = max(t[0] for t in terms) + 1
    NW = max(t[1] for t in terms) + 1
    NT = T * BL
    chunks = []
    t0 = 0
    while t0 < T:
        tcn = min(CH, T - t0)
        chunks.append((t0, tcn))
        t0 += tcn

    nc = bacc.Bacc("TRN2", target_bir_lowering=False, debug=False)
    xt_d = nc.dram_tensor("xt", [NX, C, IN, NT], F16, kind="ExternalInput").ap()
    wt_d = nc.dram_tensor("wt", [C, NW, IN, DHP], F16, kind="ExternalInput").ap()
    bh_d = nc.dram_tensor("bh", [128, NJ], F32, kind="ExternalInput").ap()
    wmm_d = nc.dram_tensor("wmm", [128, NJ, NOUT], F16, kind="ExternalInput").ap()
    vout_d = nc.dram_tensor("vout", [NOUT, NT], F32, kind="ExternalOutput").ap()

    with tile.TileContext(nc) as tc:
        with ExitStack() as ctx:
            const_p = ctx.enter_context(tc.tile_pool(name="const", bufs=1))
            state_p = ctx.enter_context(tc.tile_pool(name="state", bufs=1))
            xc_p = ctx.enter_context(tc.tile_pool(name="xc", bufs=2))
            injc_p = ctx.enter_context(tc.tile_pool(name="injc", bufs=2))
            maskc_p = ctx.enter_context(tc.tile_pool(name="maskc", bufs=1))
            wtmp_p = ctx.enter_context(tc.tile_pool(name="wtmp", bufs=1))
            psA_p = ctx.enter_context(tc.tile_pool(name="psA", bufs=4, space="PSUM"))
            psP_p = ctx.enter_context(tc.tile_pool(name="psP", bufs=2, space="PSUM"))
            pallc_p = ctx.enter_context(tc.tile_pool(name="pallc", bufs=2))

            w_sbs = []
            for c in range(C):
                row = []
                for wi in range(NW):
                    wt_t = const_p.tile([128, NK, NM, 128], F16, tag=f"w{c}{wi}")
                    nc.sync.dma_start(
                        wt_t[:],
                        wt_d[c, wi].rearrange("(k p) (m q) -> p k m q", p=128, q=128),
                    )
                    row.append(wt_t)
                w_sbs.append(row)
            bh_sb = const_p.tile([128, NJ], F32)
            nc.sync.dma_start(bh_sb[:], bh_d[:])
            wmm_sb = const_p.tile([128, NJ, NOUT], F16)
            nc.sync.dma_start(wmm_sb[:], wmm_d[:])
            dec8_sb = const_p.tile([NOUT, T], F32)
            nc.vector.memset(dec8_sb[:], 0.8)
            dec9_sb = const_p.tile([NOUT, T], F32)
            nc.vector.memset(dec9_sb[:], 0.9)

            u_sb = state_p.tile([128, NJ, BL], F32)
            ih_sb = state_p.tile([128, NJ, BL], F32)
            abuf = state_p.tile([NOUT, NT + BL], F32)
            vout_sb = state_p.tile([NOUT, NT], F32)
            nc.vector.memset(u_sb[:], 0.0)
            nc.vector.memset(ih_sb[:], 0.0)
            nc.vector.memset(abuf[:, 0:BL], 0.0)

            for (t0, tcn) in chunks:
                CW = tcn * BL
                injt = injc_p.tile([128, NJ, CH * BL], F32, tag="injc")
                maskt = maskc_p.tile([128, CH, NJ, BL], F16, tag="maskc")
                for c in range(C):
                    xtile = xc_p.tile([128, NX, NK, CH * BL], F16, tag="xc")
                    for xi in range(NX):
                        nc.sync.dma_start(
                            xtile[:, xi, :, 0:CW],
                            xt_d[xi, c].rearrange("(k p) n -> p k n", p=128)[
                                :, :, t0 * BL : t0 * BL + CW
                            ],
                        )
                    for m in range(NM):
                        ps = psA_p.tile([128, CH * BL], F32, tag="psA")
                        nmm = len(terms) * NK
                        i_mm = 0
                        for (xi, wi) in terms:
                            for k in range(NK):
                                nc.tensor.matmul(
                                    ps[:, 0:CW],
                                    w_sbs[c][wi][:, k, m, :],
                                    xtile[:, xi, k, 0:CW],
                                    start=(i_mm == 0),
                                    stop=(i_mm == nmm - 1),
                                )
                                i_mm += 1
                        j = c * NM + m
                        nc.scalar.activation(
                            injt[:, j, 0:CW],
                            ps[:, 0:CW],
                            mybir.ActivationFunctionType.Identity,
                            bias=bh_sb[:, j : j + 1],
                        )
                for tt in range(tcn):
                    inj_sl = injt[:, :, tt * BL : (tt + 1) * BL]
                    nc.vector.scalar_tensor_tensor(
                        ih_sb[:], ih_sb[:], 0.8, inj_sl,
                        mybir.AluOpType.mult, mybir.AluOpType.add,
                    )
                    nc.vector.scalar_tensor_tensor(
                        maskt[:, tt], u_sb[:], 10.0, u_sb[:],
                        mybir.AluOpType.is_gt, mybir.AluOpType.bypass,
                    )
                    w_t = wtmp_p.tile([128, NJ, BL], F32, tag="wtmp")
                    nc.vector.scalar_tensor_tensor(
                        w_t[:], u_sb[:], 10.0, u_sb[:],
                        mybir.AluOpType.is_le, mybir.AluOpType.mult,
                    )
                    nc.vector.scalar_tensor_tensor(
                        u_sb[:], w_t[:], 0.9, ih_sb[:],
                        mybir.AluOpType.mult, mybir.AluOpType.add,
                    )
                psP = psP_p.tile([NOUT, CH * BL], F32, tag="psP")
                for j in range(NJ):
                    nc.tensor.matmul(
                        psP[:, 0:CW],
                        wmm_sb[:, j, :],
                        maskt[:, 0:tcn, j, :],
                        start=(j == 0),
                        stop=(j == NJ - 1),
                    )
                Pall_c = pallc_p.tile([NOUT, CH * BL], F32, tag="pallc")
                nc.scalar.copy(Pall_c[:, 0:CW], psP[:, 0:CW])
                # incremental output IIRs for this chunk (state carried via
                # abuf/vout columns written by the previous chunk)
                Pall_bt = Pall_c.rearrange("n (t b) -> n b t", b=BL)
                aw_bt = abuf[:, BL : BL + NT].rearrange("n (t b) -> n b t", b=BL)
                ar_bt = abuf[:, 0:NT].rearrange("n (t b) -> n b t", b=BL)
                vout_bt = vout_sb.rearrange("n (t b) -> n b t", b=BL)
                ts_sl = slice(t0, t0 + tcn)
                for b in range(BL):
                    nc.vector.tensor_tensor_scan(
                        aw_bt[:, b, ts_sl], dec8_sb[:, ts_sl], Pall_bt[:, b, 0:tcn],
                        abuf[:, t0 * BL + b : t0 * BL + b + 1],
                        mybir.AluOpType.mult, mybir.AluOpType.add,
                    )
                for b in range(BL):
                    init = (0.0 if t0 == 0 else
                            vout_sb[:, (t0 - 1) * BL + b : (t0 - 1) * BL + b + 1])
                    nc.vector.tensor_tensor_scan(
                        vout_bt[:, b, ts_sl], dec9_sb[:, ts_sl], ar_bt[:, b, ts_sl],
                        init,
                        mybir.AluOpType.mult, mybir.AluOpType.add,
                    )
                nc.sync.dma_start(
                    vout_d[:, t0 * BL : t0 * BL + CW],
                    vout_sb[:, t0 * BL : t0 * BL + CW],
                )
    nc.compile()
    return nc


def _prep_weights(W_h, b_h, W_o, b_o, nterms=NTERMS):
    NW = 2 if nterms >= 3 else 1
    W_hi = W_h.astype(np.float16)
    W_lo = (W_h.astype(np.float32) - W_hi.astype(np.float32)).astype(np.float16)
    wt = np.zeros((C, NW, IN, DHP), np.float16)
    for wi, W in enumerate([W_hi, W_lo][:NW]):
        wt[:, wi, :, : D * H] = W.reshape(C, D * H, IN).transpose(0, 2, 1)
    O = W_o.shape[0]
    K = H // O
    # per-c padded cdh' layout: [c, m*128+p] with dh = m*128+p < 600 valid
    bh_p = np.zeros((C, DHP), np.float32)
    bh_p[:, : D * H] = b_h.reshape(C, D * H)
    bh = bh_p.reshape(C * NM, 128).T.copy()  # [128, NJ]
    h_of_dh = np.arange(D * H) % H
    wz = (0.1 * W_o.transpose(0, 2, 1).reshape(H, NOUT)[h_of_dh]).astype(np.float16)
    wmm_p = np.zeros((C, DHP, NOUT), np.float16)
    wmm_p[:, : D * H] = wz[None]
    wmm = np.ascontiguousarray(
        wmm_p.reshape(C * NM, 128, NOUT).transpose(1, 0, 2)
    )  # [128, NJ, NOUT]
    K_n = (0.1 * b_o.sum(axis=0)).astype(np.float32)
    return wt, bh, wmm, K_n


def _host_A(K_n, T=T):
    aio = np.zeros(NOUT, np.float32)
    avo = np.zeros(NOUT, np.float32)
    A = np.zeros((T, NOUT), np.float32)
    for t in range(T):
        avo = (np.float32(0.9) * avo + aio).astype(np.float32)
        A[t] = avo
        aio = (np.float32(0.8) * aio + K_n).astype(np.float32)
    return A


def _prep_x_core(x_core, nterms=NTERMS):
    Tl = x_core.shape[0]
    NX = 2 if nterms >= 2 else 1
    xf = np.ascontiguousarray(x_core.reshape(Tl, BL, C, IN))
    x_hi = xf.astype(np.float16)
    parts = [x_hi]
    if NX == 2:
        x_lo = (xf - x_hi.astype(np.float32)).astype(np.float16)
        parts.append(x_lo)
    xt = np.empty((NX, C, IN, Tl * BL), np.float16)
    for xi, xp in enumerate(parts):
        xt[xi] = xp.transpose(2, 3, 0, 1).reshape(C, IN, Tl * BL)
    return xt


_CACHED_NC = None


def run_on_device(x, W_h, b_h, W_o, b_o, trace=False):
    global _CACHED_NC
    x = np.asarray(x, np.float32)
    W_h = np.asarray(W_h, np.float32)
    b_h = np.asarray(b_h, np.float32)
    W_o = np.asarray(W_o, np.float32)
    b_o = np.asarray(b_o, np.float32)
    wt, bh, wmm, K_n = _prep_weights(W_h, b_h, W_o, b_o)
    A = _host_A(K_n)
    in_maps = []
    for core in range(NCORES):
        xt = _prep_x_core(x[:, core * BL : (core + 1) * BL])
        in_maps.append({"xt": xt, "wt": wt, "bh": bh, "wmm": wmm})
    if _CACHED_NC is None:
        _CACHED_NC = _build()
    res = run_bass_kernel_spmd(
        _CACHED_NC, in_maps, core_ids=list(range(NCORES)), trace=trace
    )
    out = np.empty((T, B, NOUT), np.float32)
    for core in range(NCORES):
        v = res.results[core]["vout"]
        out[:, core * BL : (core + 1) * BL, :] = (
            v.reshape(NOUT, T, BL).transpose(1, 2, 0)
        )
    out += A[:, None, :]
    return out, res.exec_time_ns


def kernel(x, W_h, b_h, W_o, b_o):
    out, _ = run_on_device(x, W_h, b_h, W_o, b_o, trace=False)
    return out
